# revision 1
# baseline (speedup 1.0000x reference)
"""Trainium2 Bass kernel for distance-based (RBF) attention.

Reference computation (per batch b):
    Q = x @ Wq.T           (N, 64)
    K = x @ Wk.T           (N, 64)
    V = x @ Wv.T           (N, 512)
    dist2[i,j] = |Q_i - K_j|^2
    attn = softmax(-dist2 / (2 lam^2), axis=-1)
    out = attn @ V

Key identity used: softmax_j(-(q_i^2 + k_j^2 - 2 q_i.k_j)/(2 lam^2)) ==
softmax_j((q_i.k_j - k_j^2/2) / lam^2)  -- the per-row q_i^2 term is a
row-constant and cancels; exp without max-subtraction is safe because the
dominant terms are far inside fp32 range and the normalizer divides the
common scale back out.

Sharding: 8 cores = 4 batches x 2 query-halves. Each core computes the
full K/V projections for its batch (keys order = [its half, other half];
softmax is permutation-invariant over keys) and attention output for its
2048 query rows. No cross-core communication.
"""

import numpy as np
from contextlib import ExitStack

import concourse.bacc as bacc
import concourse.tile as tile
import concourse.mybir as mybir
from concourse import masks
from concourse.bass_utils import run_bass_kernel_spmd

P = 128
D = 64          # head dim
IN_F = 512
OUT_F = 512
NQ = 2048       # query rows per core
NK = 4096       # keys per core (full batch)
N_CORES = 8
F32 = mybir.dt.float32
F32R = mybir.dt.float32r
BF16 = mybir.dt.bfloat16
AF = mybir.ActivationFunctionType

X_TRANSPOSE = "pe"  # "pe" or "dma"
L_MODE = "dve"       # "pe" (ones-matmuls) or "dve" (lsum adds)
PV_DTYPE = "f32r"    # "f32r" or "bf16": dtype of pt/V operands of the PV matmul

LAST_RESULTS = None  # test harness reads exec_time_ns from here
_LAST_NC = None
_LAST_IN_MAPS = None


def timed_rerun(n=3):
    """Re-execute the last compiled program; returns list of wall times (s)."""
    import time

    times = []
    for _ in range(n):
        t0 = time.perf_counter()
        run_bass_kernel_spmd(_LAST_NC, _LAST_IN_MAPS, list(range(N_CORES)))
        times.append(time.perf_counter() - t0)
    return times


def build_program(lam: float):
    nc = bacc.Bacc(
        "TRN2", target_bir_lowering=False, debug=False, num_devices=N_CORES
    )
    xq = nc.dram_tensor("xq", [NQ, IN_F], F32, kind="ExternalInput").ap()
    xo = nc.dram_tensor("xo", [NQ, IN_F], F32, kind="ExternalInput").ap()
    wq = nc.dram_tensor("wq", [D, IN_F], F32, kind="ExternalInput").ap()
    wk = nc.dram_tensor("wk", [D, IN_F], F32, kind="ExternalInput").ap()
    wv = nc.dram_tensor("wv", [OUT_F, IN_F], F32, kind="ExternalInput").ap()
    out = nc.dram_tensor("out", [NQ, OUT_F], F32, kind="ExternalOutput").ap()

    inv2 = 1.0 / (lam * lam)
    neghalf = -0.5 * inv2

    with tile.TileContext(nc) as tc, ExitStack() as octx:
        # ---------- long-lived pools ----------
        cpool = octx.enter_context(tc.tile_pool(name="const", bufs=1))
        ident = cpool.tile([P, P], F32, tag="ident")
        masks.make_identity(nc, ident[:])
        identr = cpool.tile([P, P], F32R, tag="identr")
        nc.vector.tensor_copy(identr[:], ident[:])
        pv_dt = BF16 if PV_DTYPE == "bf16" else F32R
        tmp1 = cpool.tile([P, 2], F32, tag="tmp1")
        nc.vector.memset(tmp1[:], 1.0)
        ones2 = cpool.tile([P, 2], F32R, tag="ones2")
        nc.vector.tensor_copy(ones2[:], tmp1[:])
        ones2pv = cpool.tile([P, 2], pv_dt, tag="ones2pv")
        nc.vector.tensor_copy(ones2pv[:], tmp1[:])
        tmpn = cpool.tile([D, 2], F32, tag="tmpn")
        nc.vector.memset(tmpn[:], neghalf)
        negh64 = cpool.tile([D, 2], F32R, tag="negh64")
        nc.vector.tensor_copy(negh64[:], tmpn[:])

        kt_pool = octx.enter_context(tc.tile_pool(name="kt", bufs=1))
        KT = kt_pool.tile([D, NK], F32R, tag="KT")
        qt_pool = octx.enter_context(tc.tile_pool(name="qt", bufs=1))
        QT = qt_pool.tile([D, NQ], F32R, tag="QT")
        v_pool = octx.enter_context(tc.tile_pool(name="v", bufs=1))
        V = [v_pool.tile([P, OUT_F], pv_dt, tag=f"V{j}", name=f"V{j}")
             for j in range(32)]
        bias_pool = octx.enter_context(tc.tile_pool(name="biask", bufs=1))
        biask = [bias_pool.tile([P, 1], F32, tag=f"bk{j}", name=f"bk{j}")
                 for j in range(32)]

        # ---- phase A+B: load x, convert to bf16, DMA-transpose, project ----
        with ExitStack() as pctx:
            xload = pctx.enter_context(tc.tile_pool(name="xload", bufs=6))
            xbf_pool = pctx.enter_context(tc.tile_pool(name="xbf", bufs=4))
            xT_pool = pctx.enter_context(tc.tile_pool(name="xT", bufs=1))
            xT = [xT_pool.tile([P, NK], F32R, tag=f"xT{c}", name=f"xT{c}")
                  for c in range(4)]
            wpool = pctx.enter_context(tc.tile_pool(name="w", bufs=1))
            wT_pool = pctx.enter_context(tc.tile_pool(name="wT", bufs=1))
            sq_pool = pctx.enter_context(tc.tile_pool(name="sq", bufs=1))
            tpsum = pctx.enter_context(
                tc.tile_pool(name="tpsum", bufs=4, space="PSUM")
            )
            projpsum = pctx.enter_context(
                tc.tile_pool(name="projpsum", bufs=2, space="PSUM")
            )
            kpsum = pctx.enter_context(
                tc.tile_pool(name="kpsum", bufs=1, space="PSUM")
            )

            # load + transpose weights on PE (small)
            wq_sb = wpool.tile([D, IN_F], F32, tag="wq_sb")
            nc.sync.dma_start(wq_sb[:], wq)
            wk_sb = wpool.tile([D, IN_F], F32, tag="wk_sb")
            nc.sync.dma_start(wk_sb[:], wk)
            wv_sb = [wpool.tile([P, IN_F], F32, tag=f"wv_sb{i}", name=f"wv_sb{i}")
                     for i in range(4)]
            for i in range(4):
                nc.sync.dma_start(wv_sb[i][:], wv[i * P : (i + 1) * P, :])

            wqT = [wT_pool.tile([P, D], F32R, tag=f"wqT{c}", name=f"wqT{c}")
                   for c in range(4)]
            wkT = [wT_pool.tile([P, D], F32R, tag=f"wkT{c}", name=f"wkT{c}")
                   for c in range(4)]
            wvT = [wT_pool.tile([P, OUT_F], F32R, tag=f"wvT{c}", name=f"wvT{c}")
                   for c in range(4)]
            for c in range(4):
                tp = tpsum.tile([P, P], F32, tag="tp")
                nc.tensor.transpose(
                    tp[:, :D], wq_sb[:, c * P : (c + 1) * P], ident[:D, :D]
                )
                nc.vector.tensor_copy(wqT[c][:], tp[:, :D])
                tp = tpsum.tile([P, P], F32, tag="tp")
                nc.tensor.transpose(
                    tp[:, :D], wk_sb[:, c * P : (c + 1) * P], ident[:D, :D]
                )
                nc.vector.tensor_copy(wkT[c][:], tp[:, :D])
            for fc in range(4):
                for oc in range(4):
                    tp = tpsum.tile([P, P], F32, tag="tp")
                    nc.tensor.transpose(
                        tp[:], wv_sb[oc][:, fc * P : (fc + 1) * P], ident[:]
                    )
                    nc.vector.tensor_copy(
                        wvT[fc][:, oc * P : (oc + 1) * P], tp[:]
                    )

            # x into xT bf16 (keys order: [xq rows, xo rows])
            for si, src in enumerate((xq, xo)):
                soff = si * NQ
                for t in range(NQ // P):
                    xt = xload.tile([P, IN_F], F32R, tag="xload")
                    nc.sync.dma_start(xt[:], src[t * P : (t + 1) * P, :].bitcast(F32R))
                    col = soff + t * P
                    if X_TRANSPOSE == "dma":
                        xb = xbf_pool.tile([P, IN_F], BF16, tag="xbf")
                        nc.scalar.copy(xb[:], xt[:])
                        for fc in range(4):
                            nc.sync.dma_start(
                                xT[fc][:, col : col + P],
                                xb[:, fc * P : (fc + 1) * P],
                                transpose=True,
                            )
                    else:
                        for fc in range(4):
                            tp = tpsum.tile([P, P], F32R, tag="tp")
                            nc.tensor.transpose(
                                tp[:], xt[:, fc * P : (fc + 1) * P], identr[:]
                            )
                            nc.vector.tensor_copy(xT[fc][:, col : col + P], tp[:])

            # KT = Wk @ x^T  [64, 4096]
            for nb in range(NK // 512):
                pp = projpsum.tile([D, 512], F32, tag="pp", name="pp")
                for fc in range(4):
                    nc.tensor.matmul(
                        pp[:],
                        wkT[fc][:],
                        xT[fc][:, nb * 512 : (nb + 1) * 512],
                        start=(fc == 0),
                        stop=(fc == 3),
                    )
                nc.vector.tensor_copy(KT[:, nb * 512 : (nb + 1) * 512], pp[:])

            # QT = Wq @ xq^T  [64, 2048]  (xT cols 0:2048 are xq rows)
            for nb in range(NQ // 512):
                pp = projpsum.tile([D, 512], F32, tag="pp", name="pp")
                for fc in range(4):
                    nc.tensor.matmul(
                        pp[:],
                        wqT[fc][:],
                        xT[fc][:, nb * 512 : (nb + 1) * 512],
                        start=(fc == 0),
                        stop=(fc == 3),
                    )
                nc.vector.tensor_copy(QT[:, nb * 512 : (nb + 1) * 512], pp[:])

            # V = x @ Wv^T  as 32 tiles [128, 512]
            for jc in range(32):
                pv = projpsum.tile([P, OUT_F], F32, tag="pp", name="pv")
                for fc in range(4):
                    nc.tensor.matmul(
                        pv[:],
                        xT[fc][:, jc * P : (jc + 1) * P],
                        wvT[fc][:],
                        start=(fc == 0),
                        stop=(fc == 3),
                    )
                nc.vector.tensor_copy(V[jc][:], pv[:])

            # biask[jc][j,0] = -|K_j|^2 / (2 lam^2), per 128-key chunk
            for nb in range(NK // 512):
                sq = sq_pool.tile([D, 512], F32R, tag="sq", bufs=2)
                nc.vector.tensor_mul(
                    sq[:], KT[:, nb * 512 : (nb + 1) * 512],
                    KT[:, nb * 512 : (nb + 1) * 512],
                )
                kp = kpsum.tile([2, 512], F32, tag="kp")
                nc.tensor.matmul(kp[:], negh64[:], sq[:], start=True, stop=True)
                krow = sq_pool.tile([1, 512], F32, tag="krow", bufs=2)
                nc.vector.tensor_copy(krow[:], kp[0:1, :])
                for j in range(4):
                    jc = nb * 4 + j
                    tb = tpsum.tile([P, P], F32, tag="tp")
                    nc.tensor.transpose(
                        tb[:, 0:1], krow[:, j * P : (j + 1) * P], ident[0:1, 0:1]
                    )
                    nc.vector.tensor_copy(biask[jc][:], tb[:, 0:1])

        # ---------- phase C: attention ----------
        with ExitStack() as actx:
            spsum = actx.enter_context(
                tc.tile_pool(name="spsum", bufs=4, space="PSUM")
            )
            opsum = actx.enter_context(
                tc.tile_pool(name="opsum", bufs=1, space="PSUM")
            )
            ptpool = actx.enter_context(tc.tile_pool(name="pt", bufs=4))
            onpool = actx.enter_context(tc.tile_pool(name="on", bufs=4))
            recpool = actx.enter_context(tc.tile_pool(name="rec", bufs=3))

            for ib in range(NQ // 512):
                outp = [opsum.tile([P, OUT_F], F32, tag=f"op{i}", name=f"op{ib}_{i}")
                        for i in range(4)]
                lp = None
                if L_MODE == "pe":
                    lp = spsum.tile([P, 8], F32, tag="sp", name=f"lp{ib}")
                lsums = [None, None]
                for jc in range(32):
                    sp = spsum.tile([P, 512], F32, tag="sp")
                    nc.tensor.matmul(
                        sp[:],
                        KT[:, jc * P : (jc + 1) * P],
                        QT[:, ib * 512 : (ib + 1) * 512],
                        start=True,
                        stop=True,
                    )
                    pt = ptpool.tile([P, 512], pv_dt, tag="pt")
                    nc.scalar.activation(
                        pt[:], sp[:], AF.Exp, bias=biask[jc][:], scale=inv2
                    )
                    if L_MODE == "dve":
                        if jc < 2:
                            lsums[jc] = ptpool.tile(
                                [P, 512], F32R, tag=f"lsum{jc}",
                                name=f"lsum{ib}_{jc}", bufs=2,
                            )
                            nc.vector.tensor_copy(lsums[jc][:], pt[:])
                        else:
                            a = lsums[jc % 2]
                            nc.vector.tensor_add(a[:], a[:], pt[:])
                    for ic in range(4):
                        nc.tensor.matmul(
                            outp[ic][:],
                            pt[:, ic * P : (ic + 1) * P],
                            V[jc][:],
                            start=(jc == 0),
                            stop=(jc == 31),
                        )
                        if L_MODE == "pe":
                            nc.tensor.matmul(
                                lp[:, 2 * ic : 2 * ic + 2],
                                pt[:, ic * P : (ic + 1) * P],
                                ones2pv[:],
                                start=(jc == 0),
                                stop=(jc == 31),
                            )
                if L_MODE == "dve":
                    nc.vector.tensor_add(lsums[0][:], lsums[0][:], lsums[1][:])
                    lp = spsum.tile([P, 8], F32, tag="sp", name=f"lp{ib}")
                    for ic in range(4):
                        nc.tensor.matmul(
                            lp[:, 2 * ic : 2 * ic + 2],
                            lsums[0][:, ic * P : (ic + 1) * P],
                            ones2[:],
                            start=True,
                            stop=True,
                        )
                rec = recpool.tile([P, 8], F32, tag="rec")
                nc.vector.reciprocal(rec[:], lp[:])
                for ic in range(4):
                    on = onpool.tile([P, OUT_F], F32, tag="on")
                    nc.vector.tensor_scalar_mul(
                        on[:], outp[ic][:], rec[:, 2 * ic : 2 * ic + 1]
                    )
                    r0 = ib * 512 + ic * P
                    nc.sync.dma_start(out[r0 : r0 + P, :], on[:])

    nc.compile()
    return nc


_CACHE = {}


def _get_program(lam: float):
    key = round(float(lam), 9)
    if key not in _CACHE:
        _CACHE[key] = build_program(key)
    return _CACHE[key]


def kernel(x, Wq, Wk, Wv, log_lambda):
    x = np.asarray(x, dtype=np.float32)
    Wq = np.ascontiguousarray(np.asarray(Wq, dtype=np.float32))
    Wk = np.ascontiguousarray(np.asarray(Wk, dtype=np.float32))
    Wv = np.ascontiguousarray(np.asarray(Wv, dtype=np.float32))
    lam = float(np.clip(np.exp(np.asarray(log_lambda, np.float32)[0]), 1e-3, None))

    nc = _get_program(lam)

    in_maps = []
    for c in range(N_CORES):
        b, h = divmod(c, 2)
        xb = x[b]
        xq_ = np.ascontiguousarray(xb[h * NQ : (h + 1) * NQ])
        xo_ = np.ascontiguousarray(xb[(1 - h) * NQ : (2 - h) * NQ])
        in_maps.append({"xq": xq_, "xo": xo_, "wq": Wq, "wk": Wk, "wv": Wv})

    res = run_bass_kernel_spmd(nc, in_maps, list(range(N_CORES)))
    global LAST_RESULTS, _LAST_NC, _LAST_IN_MAPS
    LAST_RESULTS = res
    _LAST_NC = nc
    _LAST_IN_MAPS = in_maps

    out = np.empty((4, 2 * NQ, OUT_F), np.float32)
    for c in range(N_CORES):
        b, h = divmod(c, 2)
        out[b, h * NQ : (h + 1) * NQ] = res.results[c]["out"]
    return out

